# revision 17
# baseline (speedup 1.0000x reference)
"""Trainium2 Bass kernel for an 8-expert top-2 MoE layer (B=4, T=2048, C=1024,
F=4096), expert-parallel across 8 NeuronCores — quarter-slot balanced variant.

Routing/pairing
---------------
Host computes the fp32 gate and routes each token to its top-2 experts.  The
16 (expert, F-half) jobs of the paired scheme padded every core to
max(bigs)+max(smalls) = 4204 token-half-units (ideal 4096).  Here each core
instead runs FOUR slots at F-QUARTER depth.  Sorted by count, experts are
paired (E1,E2),(E3,E4),(E5,E6),(E7,E8); slot s is compiled for
S_s = max count of pair s, and core i's slot s holds (expert = pair_s[i//4],
F-quarter = i%4).  Every expert-quarter combo lands on exactly one core, and
per-core padded work drops to Sigma S_s = 4139 half-units.

Host sums the 4 per-quarter partial outputs of each expert and scatter-adds
with the gate weights (plus w*b2, which never goes to device).

On-device per slot (expert e, quarter q), streaming token chunks:
    hT[f, t]   = sum_c W1[c, f] * xT[c, t]     f in quarter q   (PE bf16)
    hT         = gelu_erf(hT + b1[f])          (ScalarE, fused bias)
    out[t, cc] = sum_f h[t, f] * W2[f, cc]     (PE bf16, fp32 acc)
out ships bf16.  Startup: scratch-matmul HAM warmup bridging the NEFF
preamble to the first chunk's (contiguous) DMAs; all DMA is one FIFO HW
queue so issue order == service order.
"""

import os

import numpy as np
import ml_dtypes

import concourse.bass as bass
import concourse.mybir as mybir
import concourse.tile as tile
from concourse import bacc
from concourse.bass_utils import run_bass_kernel_spmd

C = 1024
F = 4096
FQ = F // 4  # per-slot F quarter (1024)
E = 8
K = 2
N_CORES = 8
N_SLOTS = 4

BF16 = mybir.dt.bfloat16
F32 = mybir.dt.float32

N_CT = C // 128  # 8 contraction tiles for x @ W1
N_FT = FQ // 128  # 8 F tiles per quarter
N_CC = C // 512  # 2 output column chunks


def build_nc(chunk_lists: list[list[int]]) -> bass.Bass:
    """Bass program: four expert-quarter FFN slots over their token chunks."""
    assert len(chunk_lists) == N_SLOTS
    nts = [sum(chs) for chs in chunk_lists]
    nc = bacc.Bacc(None)

    # per-slot tensors; x pre-swizzled [128, 8, nt]: (p, chi, t) = xT[chi*128+p, t]
    x0 = nc.dram_tensor("x0", [128, N_CT, chunk_lists[0][0]], BF16,
                        kind="ExternalInput")  # contiguous startup copy
    xs = [nc.dram_tensor(f"x{s}t", [128, N_CT, nts[s]], BF16, kind="ExternalInput")
          for s in range(N_SLOTS)]
    # W1 pieces [4, 128, 8, FQ/4]: (j, p, chi, f) = W1[chi*128+p,
    # fq0 + j*(FQ/4) + f] — each piece a contiguous 0.5MB read, so the
    # startup-critical first piece lands (and unblocks MM1 f0-f1) sooner.
    w1s = [nc.dram_tensor(f"w1{s}", [4, 128, N_CT, FQ // 4], BF16,
                          kind="ExternalInput") for s in range(N_SLOTS)]
    # W2 halves [2, 128, 4, C]: (h, p, fhi, c) = W2[fq0 + (4h+fhi)*128 + p, c]
    w2s = [nc.dram_tensor(f"w2{s}", [2, 128, N_FT // 2, C], BF16,
                          kind="ExternalInput") for s in range(N_SLOTS)]
    b1t = nc.dram_tensor("b1t", [N_SLOTS, 128, N_FT], F32, kind="ExternalInput")
    outs = [nc.dram_tensor(f"out{s}", [nts[s], C], BF16, kind="ExternalOutput")
            for s in range(N_SLOTS)]

    with tile.TileContext(nc) as tc:
        with (
            tc.tile_pool(name="wpool", bufs=1) as wpool,
            tc.tile_pool(name="xpool", bufs=4) as xpool,
            tc.tile_pool(name="hpool", bufs=N_FT + 2) as hpool,
            tc.tile_pool(name="opool", bufs=4) as opool,
            tc.tile_pool(name="phpool", bufs=3, space="PSUM") as phpool,
            tc.tile_pool(name="popool", bufs=5, space="PSUM") as popool,
        ):
            # HAM warmup: bridge the NEFF preamble (~7us) to the first chunk's
            # DMA arrival (~12us) with scratch matmuls so the PE clock gate
            # opens and stays open.  13 N=512 MMs at the cold clock span
            # ~5.2us; overshooting queues real MMs behind scratch work.
            warm_w = wpool.tile([128, 512], BF16, name="warm_w", tag="warmw")
            nc.gpsimd.memset(warm_w, 0)
            warm_ps = popool.tile([128, 512], F32, name="warm_ps", tag="po")
            for _ in range(13):
                nc.tensor.matmul(warm_ps[:64, :], lhsT=warm_w[:, :64], rhs=warm_w,
                                 start=True, stop=True)

            w1_sb = [wpool.tile([128, N_CT, FQ], BF16, name=f"w1sb{s}", tag=f"w1sb{s}")
                     for s in range(N_SLOTS)]
            w2_sb = [wpool.tile([128, N_FT, C], BF16, name=f"w2sb{s}", tag=f"w2sb{s}")
                     for s in range(N_SLOTS)]
            b1_sb = wpool.tile([128, N_SLOTS, N_FT], F32, name="b1sb", tag="b1sb")

            def load_w1(s, j):
                nc.sync.dma_start(
                    out=w1_sb[s][:, :, j * (FQ // 4) : (j + 1) * (FQ // 4)],
                    in_=w1s[s][j],
                )

            def load_w2(s, h):
                nc.sync.dma_start(
                    out=w2_sb[s][:, 4 * h : 4 * h + 4, :], in_=w2s[s][h]
                )

            # startup: slot-0 criticals in consumption order
            load_w1(0, 0)
            xts = {}
            xts[(0, 0)] = xpool.tile(
                [128, N_CT, chunk_lists[0][0]], BF16, name="x0_0", tag="xt"
            )
            nc.sync.dma_start(out=xts[(0, 0)], in_=x0[:, :, :])
            for s in range(N_SLOTS):
                nc.sync.dma_start(out=b1_sb[:, s, :], in_=b1t[s])
            for j in (1, 2, 3):
                load_w1(0, j)
            load_w2(0, 0)
            load_w2(0, 1)

            # later slots' weights: drip between earlier chunks (FIFO queue)
            deferred = []
            for s in range(1, N_SLOTS):
                deferred.append([lambda s=s: load_w1(s, 0),
                                 lambda s=s: load_w1(s, 1)])
                deferred.append([lambda s=s: load_w1(s, 2),
                                 lambda s=s: load_w1(s, 3)])
                deferred.append([lambda s=s: load_w2(s, 0),
                                 lambda s=s: load_w2(s, 1)])

            def run_slot(s):
                chunks = chunk_lists[s]
                tok0 = 0
                for tk, ch in enumerate(chunks):
                    if (s, tk) in xts:
                        xt = xts[(s, tk)]
                    else:
                        xt = xpool.tile(
                            [128, N_CT, ch], BF16, name=f"xt{s}_{tk}", tag="xt"
                        )
                        nc.sync.dma_start(out=xt, in_=xs[s][:, :, tok0 : tok0 + ch])
                    if deferred and not (s == 0 and tk == 0):
                        for emit in deferred.pop(0):
                            emit()

                    hts = []
                    for f in range(N_FT):
                        ph = phpool.tile([128, ch], F32, name=f"ph{s}_{tk}_{f}", tag="ph")
                        for c in range(N_CT):
                            nc.tensor.matmul(
                                ph,
                                lhsT=w1_sb[s][:, c, f * 128 : (f + 1) * 128],
                                rhs=xt[:, c, :],
                                start=(c == 0),
                                stop=(c == N_CT - 1),
                            )
                        ht = hpool.tile([128, ch], BF16, name=f"ht{s}_{tk}_{f}", tag="ht")
                        nc.scalar.activation(
                            out=ht,
                            in_=ph,
                            func=mybir.ActivationFunctionType.Gelu,
                            bias=b1_sb[:, s, f : f + 1],
                            scale=1.0,
                        )
                        hts.append(ht)

                    for tt in range((ch + 127) // 128):
                        tw = min(128, ch - tt * 128)
                        for cc in range(N_CC):
                            po = popool.tile(
                                [128, 512], F32, name=f"po{s}_{tk}_{tt}_{cc}", tag="po"
                            )
                            for f in range(N_FT):
                                nc.tensor.matmul(
                                    po[:tw, :],
                                    lhsT=hts[f][:, tt * 128 : tt * 128 + tw],
                                    rhs=w2_sb[s][:, f, cc * 512 : (cc + 1) * 512],
                                    start=(f == 0),
                                    stop=(f == N_FT - 1),
                                )
                            ot = opool.tile(
                                [128, 512], BF16, name=f"ot{s}_{tk}_{tt}_{cc}", tag="ot"
                            )
                            nc.vector.tensor_copy(ot[:tw, :], po[:tw, :])
                            r0 = tok0 + tt * 128
                            nc.sync.dma_start(
                                out=outs[s][r0 : r0 + tw, cc * 512 : (cc + 1) * 512],
                                in_=ot[:tw, :],
                            )
                    tok0 += ch

            for s in range(N_SLOTS):
                run_slot(s)
            while deferred:
                for emit in deferred.pop(0):
                    emit()
    nc.finalize()
    return nc


def pick_chunks(n: int, small_first: bool) -> list[int]:
    """Chunks <=512 summing to n with sum(ceil(ch/128)) = ceil(n/128); a tail
    <128 borrows 128 from the previous chunk (keeps MM1 off the LDW floor)."""
    if n <= 512:
        return [n]
    chunks = []
    rem = n
    if small_first and n > 896:
        chunks.append(384)
        rem -= 384
    while rem > 512:
        chunks.append(512)
        rem -= 512
    if rem < 128 and chunks and chunks[-1] == 512:
        chunks[-1] = 384
        rem += 128
    chunks.append(rem)
    return chunks


def _route(x2d: np.ndarray, Wg: np.ndarray):
    logits = x2d @ Wg  # fp32 BLAS
    order = np.argsort(-logits, axis=1, kind="stable")
    top2 = order[:, :K]
    m = logits.max(axis=1, keepdims=True)
    p = np.exp(logits - m, dtype=np.float32)
    p /= p.sum(axis=1, keepdims=True)
    tw = np.take_along_axis(p, top2, axis=1)
    tw /= tw.sum(axis=1, keepdims=True)
    idxs, ws = [], []
    for e in range(E):
        sel = top2 == e
        rows = np.where(sel.any(axis=1))[0]
        idxs.append(rows)
        ws.append(tw[rows][sel[rows]])
    return idxs, ws


_LAST_RESULTS = {}


def kernel(**inputs: np.ndarray) -> np.ndarray:
    x = np.asarray(inputs["x"], dtype=np.float32)
    Wg = np.asarray(inputs["Wg"], dtype=np.float32)
    W1 = np.asarray(inputs["W1"], dtype=np.float32)
    b1 = np.asarray(inputs["b1"], dtype=np.float32)
    W2 = np.asarray(inputs["W2"], dtype=np.float32)
    b2 = np.asarray(inputs["b2"], dtype=np.float32)

    B, T, Cx = x.shape
    assert Cx == C
    x2d = np.ascontiguousarray(x.reshape(-1, C))
    n_tok_total = x2d.shape[0]

    idxs, ws = _route(x2d, Wg)
    counts = np.array([len(i) for i in idxs])

    # slot s serves the adjacent sorted pair (E_{2s}, E_{2s+1}); compiled
    # token count S_s = the larger of the two.  Slot 0 = smallest pair (its
    # chunk 0 is the startup critical path); slot 3 = largest pair but
    # reversed so the kernel still ends on a small tail chunk.
    order = np.argsort(-counts, kind="stable")
    slot_pairs = [
        (int(order[2 * s]), int(order[2 * s + 1])) for s in range(N_SLOTS)
    ][::-1]  # slot 0 = smallest counts, slot 3 = largest
    S = [int(max(counts[a], counts[b])) for a, b in slot_pairs]
    chunk_lists = [pick_chunks(S[s], small_first=(s == 0)) for s in range(N_SLOTS)]
    S = [sum(chs) for chs in chunk_lists]

    w1h = W1.astype(ml_dtypes.bfloat16)  # [E, C, F]
    w2h = W2.astype(ml_dtypes.bfloat16)  # [E, F, C]

    def xt_for(e, ntok):
        xe = np.zeros((ntok, C), dtype=np.float32)
        xe[: counts[e]] = x2d[idxs[e]]
        xt = xe.T.reshape(N_CT, 128, ntok).transpose(1, 0, 2)
        return np.ascontiguousarray(xt).astype(ml_dtypes.bfloat16)

    xt_cache = {}
    for s, (a, b_) in enumerate(slot_pairs):
        for e in (a, b_):
            xt_cache[e] = xt_for(e, S[s])

    in_maps = []
    for core in range(N_CORES):
        q = core % 4  # this core's F-quarter
        fsl = slice(q * FQ, (q + 1) * FQ)
        im = {}
        b1rows = []
        for s in range(N_SLOTS):
            e = slot_pairs[s][core // 4]  # expert for this core's slot s
            # W1 [2, 128, 8, FQ/2]
            w = w1h[e][:, fsl].reshape(N_CT, 128, 4, FQ // 4).transpose(2, 1, 0, 3)
            im[f"w1{s}"] = np.ascontiguousarray(w)
            # W2 [2, 128, 4, C]
            w = w2h[e][fsl, :].reshape(2, N_FT // 2, 128, C).transpose(0, 2, 1, 3)
            im[f"w2{s}"] = np.ascontiguousarray(w)
            im[f"x{s}t"] = xt_cache[e]
            b1rows.append(
                np.ascontiguousarray(b1[e][fsl].reshape(N_FT, 128).T)
            )
        im["b1t"] = np.stack(b1rows).astype(np.float32)
        im["x0"] = np.ascontiguousarray(
            im["x0t"][:, :, : chunk_lists[0][0]]
        )
        in_maps.append(im)

    nc = build_nc(chunk_lists)
    trace = os.environ.get("KERNEL_TRACE", "") == "1"
    res = run_bass_kernel_spmd(
        nc, in_maps, core_ids=list(range(N_CORES)), trace=trace
    )
    _LAST_RESULTS["bass_results"] = res
    if trace and res.exec_time_ns is not None:
        print(f"[kernel] HW exec time: {res.exec_time_ns} ns")

    out = np.zeros((n_tok_total, C), dtype=np.float32)
    for s in range(N_SLOTS):
        for half, e in enumerate(slot_pairs[s]):
            n_e = counts[e]
            oe = np.zeros((n_e, C), dtype=np.float32)
            for q in range(4):
                core = 4 * half + q
                oe += np.asarray(res.results[core][f"out{s}"])[:n_e].astype(
                    np.float32
                )
            out[idxs[e]] += ws[e][:, None] * (oe + b2[e][None, :])
    return out.reshape(B, T, C)


# revision 18
# speedup vs baseline: 1.0060x; 1.0060x over previous
"""Trainium2 Bass kernel for an 8-expert top-2 MoE layer (B=4, T=2048, C=1024,
F=4096), expert-parallel across 8 NeuronCores — quarter-slot balanced variant.

Routing/pairing
---------------
Host computes the fp32 gate and routes each token to its top-2 experts.  The
16 (expert, F-half) jobs of the paired scheme padded every core to
max(bigs)+max(smalls) = 4204 token-half-units (ideal 4096).  Here each core
instead runs FOUR slots at F-QUARTER depth.  Sorted by count, experts are
paired (E1,E2),(E3,E4),(E5,E6),(E7,E8); slot s is compiled for
S_s = max count of pair s, and core i's slot s holds (expert = pair_s[i//4],
F-quarter = i%4).  Every expert-quarter combo lands on exactly one core, and
per-core padded work drops to Sigma S_s = 4139 half-units.

Host sums the 4 per-quarter partial outputs of each expert and scatter-adds
with the gate weights (plus w*b2, which never goes to device).

On-device per slot (expert e, quarter q), streaming token chunks:
    hT[f, t]   = sum_c W1[c, f] * xT[c, t]     f in quarter q   (PE bf16)
    hT         = gelu_erf(hT + b1[f])          (ScalarE, fused bias)
    out[t, cc] = sum_f h[t, f] * W2[f, cc]     (PE bf16, fp32 acc)
out ships bf16.  Startup: scratch-matmul HAM warmup bridging the NEFF
preamble to the first chunk's (contiguous) DMAs; all DMA is one FIFO HW
queue so issue order == service order.
"""

import os

import numpy as np
import ml_dtypes

import concourse.bass as bass
import concourse.mybir as mybir
import concourse.tile as tile
from concourse import bacc
from concourse.bass_utils import run_bass_kernel_spmd

C = 1024
F = 4096
FQ = F // 4  # per-slot F quarter (1024)
E = 8
K = 2
N_CORES = 8
N_SLOTS = 4

BF16 = mybir.dt.bfloat16
F32 = mybir.dt.float32

N_CT = C // 128  # 8 contraction tiles for x @ W1
N_FT = FQ // 128  # 8 F tiles per quarter
N_CC = C // 512  # 2 output column chunks


def build_nc(chunk_lists: list[list[int]]) -> bass.Bass:
    """Bass program: four expert-quarter FFN slots over their token chunks."""
    assert len(chunk_lists) == N_SLOTS
    nts = [sum(chs) for chs in chunk_lists]
    nc = bacc.Bacc(None)

    # per-slot tensors; x pre-swizzled [128, 8, nt]: (p, chi, t) = xT[chi*128+p, t]
    x0 = nc.dram_tensor("x0", [128, N_CT, chunk_lists[0][0]], BF16,
                        kind="ExternalInput")  # contiguous startup copy
    xs = [nc.dram_tensor(f"x{s}t", [128, N_CT, nts[s]], BF16, kind="ExternalInput")
          for s in range(N_SLOTS)]
    # W1 pieces [4, 128, 8, FQ/4]: (j, p, chi, f) = W1[chi*128+p,
    # fq0 + j*(FQ/4) + f] — each piece a contiguous 0.5MB read, so the
    # startup-critical first piece lands (and unblocks MM1 f0-f1) sooner.
    w1s = [nc.dram_tensor(f"w1{s}", [4, 128, N_CT, FQ // 4], BF16,
                          kind="ExternalInput") for s in range(N_SLOTS)]
    # W2 halves [2, 128, 4, C]: (h, p, fhi, c) = W2[fq0 + (4h+fhi)*128 + p, c]
    w2s = [nc.dram_tensor(f"w2{s}", [2, 128, N_FT // 2, C], BF16,
                          kind="ExternalInput") for s in range(N_SLOTS)]
    b1t = nc.dram_tensor("b1t", [N_SLOTS, 128, N_FT], F32, kind="ExternalInput")
    outs = [nc.dram_tensor(f"out{s}", [nts[s], C], BF16, kind="ExternalOutput")
            for s in range(N_SLOTS)]

    with tile.TileContext(nc) as tc:
        with (
            tc.tile_pool(name="wpool", bufs=1) as wpool,
            tc.tile_pool(name="xpool", bufs=4) as xpool,
            tc.tile_pool(name="hpool", bufs=N_FT + 2) as hpool,
            tc.tile_pool(name="opool", bufs=4) as opool,
            tc.tile_pool(name="phpool", bufs=4, space="PSUM") as phpool,
            tc.tile_pool(name="popool", bufs=4, space="PSUM") as popool,
        ):
            # HAM warmup: bridge the NEFF preamble (~7us) to the first chunk's
            # DMA arrival (~12us) with scratch matmuls so the PE clock gate
            # opens and stays open.  13 N=512 MMs at the cold clock span
            # ~5.2us; overshooting queues real MMs behind scratch work.
            warm_w = wpool.tile([128, 512], BF16, name="warm_w", tag="warmw")
            nc.gpsimd.memset(warm_w, 0)
            warm_ps = popool.tile([128, 512], F32, name="warm_ps", tag="po")
            for _ in range(13):
                nc.tensor.matmul(warm_ps[:64, :], lhsT=warm_w[:, :64], rhs=warm_w,
                                 start=True, stop=True)

            w1_sb = [wpool.tile([128, N_CT, FQ], BF16, name=f"w1sb{s}", tag=f"w1sb{s}")
                     for s in range(N_SLOTS)]
            w2_sb = [wpool.tile([128, N_FT, C], BF16, name=f"w2sb{s}", tag=f"w2sb{s}")
                     for s in range(N_SLOTS)]
            b1_sb = wpool.tile([128, N_SLOTS, N_FT], F32, name="b1sb", tag="b1sb")

            def load_w1(s, j):
                nc.sync.dma_start(
                    out=w1_sb[s][:, :, j * (FQ // 4) : (j + 1) * (FQ // 4)],
                    in_=w1s[s][j],
                )

            def load_w2(s, h):
                nc.sync.dma_start(
                    out=w2_sb[s][:, 4 * h : 4 * h + 4, :], in_=w2s[s][h]
                )

            # startup: slot-0 criticals in consumption order
            load_w1(0, 0)
            xts = {}
            xts[(0, 0)] = xpool.tile(
                [128, N_CT, chunk_lists[0][0]], BF16, name="x0_0", tag="xt"
            )
            nc.sync.dma_start(out=xts[(0, 0)], in_=x0[:, :, :])
            for s in range(N_SLOTS):
                nc.sync.dma_start(out=b1_sb[:, s, :], in_=b1t[s])
            for j in (1, 2, 3):
                load_w1(0, j)
            load_w2(0, 0)
            load_w2(0, 1)

            # later slots' weights: drip between earlier chunks (FIFO queue)
            deferred = []
            for s in range(1, N_SLOTS):
                deferred.append([lambda s=s: load_w1(s, 0),
                                 lambda s=s: load_w1(s, 1)])
                deferred.append([lambda s=s: load_w1(s, 2),
                                 lambda s=s: load_w1(s, 3)])
                deferred.append([lambda s=s: load_w2(s, 0),
                                 lambda s=s: load_w2(s, 1)])

            def run_slot(s):
                chunks = chunk_lists[s]
                tok0 = 0
                for tk, ch in enumerate(chunks):
                    if (s, tk) in xts:
                        xt = xts[(s, tk)]
                    else:
                        xt = xpool.tile(
                            [128, N_CT, ch], BF16, name=f"xt{s}_{tk}", tag="xt"
                        )
                        nc.sync.dma_start(out=xt, in_=xs[s][:, :, tok0 : tok0 + ch])
                    if deferred and not (s == 0 and tk == 0):
                        for emit in deferred.pop(0):
                            emit()

                    hts = []
                    for f in range(N_FT):
                        ph = phpool.tile([128, ch], F32, name=f"ph{s}_{tk}_{f}", tag="ph")
                        for c in range(N_CT):
                            nc.tensor.matmul(
                                ph,
                                lhsT=w1_sb[s][:, c, f * 128 : (f + 1) * 128],
                                rhs=xt[:, c, :],
                                start=(c == 0),
                                stop=(c == N_CT - 1),
                            )
                        ht = hpool.tile([128, ch], BF16, name=f"ht{s}_{tk}_{f}", tag="ht")
                        nc.scalar.activation(
                            out=ht,
                            in_=ph,
                            func=mybir.ActivationFunctionType.Gelu,
                            bias=b1_sb[:, s, f : f + 1],
                            scale=1.0,
                        )
                        hts.append(ht)

                    for tt in range((ch + 127) // 128):
                        tw = min(128, ch - tt * 128)
                        for cc in range(N_CC):
                            po = popool.tile(
                                [128, 512], F32, name=f"po{s}_{tk}_{tt}_{cc}", tag="po"
                            )
                            for f in range(N_FT):
                                nc.tensor.matmul(
                                    po[:tw, :],
                                    lhsT=hts[f][:, tt * 128 : tt * 128 + tw],
                                    rhs=w2_sb[s][:, f, cc * 512 : (cc + 1) * 512],
                                    start=(f == 0),
                                    stop=(f == N_FT - 1),
                                )
                            ot = opool.tile(
                                [128, 512], BF16, name=f"ot{s}_{tk}_{tt}_{cc}", tag="ot"
                            )
                            nc.vector.tensor_copy(ot[:tw, :], po[:tw, :])
                            r0 = tok0 + tt * 128
                            nc.sync.dma_start(
                                out=outs[s][r0 : r0 + tw, cc * 512 : (cc + 1) * 512],
                                in_=ot[:tw, :],
                            )
                    tok0 += ch

            for s in range(N_SLOTS):
                run_slot(s)
            while deferred:
                for emit in deferred.pop(0):
                    emit()
    nc.finalize()
    return nc


def pick_chunks(n: int, small_first: bool) -> list[int]:
    """Chunks <=512 summing to n with sum(ceil(ch/128)) = ceil(n/128); a tail
    <128 borrows 128 from the previous chunk (keeps MM1 off the LDW floor)."""
    if n <= 512:
        return [n]
    chunks = []
    rem = n
    if small_first and n > 768:
        chunks.append(256)
        rem -= 256
    while rem > 512:
        chunks.append(512)
        rem -= 512
    if rem < 128 and chunks and chunks[-1] == 512:
        chunks[-1] = 384
        rem += 128
    chunks.append(rem)
    return chunks


def _route(x2d: np.ndarray, Wg: np.ndarray):
    logits = x2d @ Wg  # fp32 BLAS
    order = np.argsort(-logits, axis=1, kind="stable")
    top2 = order[:, :K]
    m = logits.max(axis=1, keepdims=True)
    p = np.exp(logits - m, dtype=np.float32)
    p /= p.sum(axis=1, keepdims=True)
    tw = np.take_along_axis(p, top2, axis=1)
    tw /= tw.sum(axis=1, keepdims=True)
    idxs, ws = [], []
    for e in range(E):
        sel = top2 == e
        rows = np.where(sel.any(axis=1))[0]
        idxs.append(rows)
        ws.append(tw[rows][sel[rows]])
    return idxs, ws


_LAST_RESULTS = {}


def kernel(**inputs: np.ndarray) -> np.ndarray:
    x = np.asarray(inputs["x"], dtype=np.float32)
    Wg = np.asarray(inputs["Wg"], dtype=np.float32)
    W1 = np.asarray(inputs["W1"], dtype=np.float32)
    b1 = np.asarray(inputs["b1"], dtype=np.float32)
    W2 = np.asarray(inputs["W2"], dtype=np.float32)
    b2 = np.asarray(inputs["b2"], dtype=np.float32)

    B, T, Cx = x.shape
    assert Cx == C
    x2d = np.ascontiguousarray(x.reshape(-1, C))
    n_tok_total = x2d.shape[0]

    idxs, ws = _route(x2d, Wg)
    counts = np.array([len(i) for i in idxs])

    # slot s serves the adjacent sorted pair (E_{2s}, E_{2s+1}); compiled
    # token count S_s = the larger of the two.  Slot 0 = smallest pair (its
    # chunk 0 is the startup critical path); slot 3 = largest pair but
    # reversed so the kernel still ends on a small tail chunk.
    order = np.argsort(-counts, kind="stable")
    slot_pairs = [
        (int(order[2 * s]), int(order[2 * s + 1])) for s in range(N_SLOTS)
    ][::-1]  # slot 0 = smallest counts, slot 3 = largest
    S = [int(max(counts[a], counts[b])) for a, b in slot_pairs]
    chunk_lists = [pick_chunks(S[s], small_first=(s == 0)) for s in range(N_SLOTS)]
    S = [sum(chs) for chs in chunk_lists]

    w1h = W1.astype(ml_dtypes.bfloat16)  # [E, C, F]
    w2h = W2.astype(ml_dtypes.bfloat16)  # [E, F, C]

    def xt_for(e, ntok):
        xe = np.zeros((ntok, C), dtype=np.float32)
        xe[: counts[e]] = x2d[idxs[e]]
        xt = xe.T.reshape(N_CT, 128, ntok).transpose(1, 0, 2)
        return np.ascontiguousarray(xt).astype(ml_dtypes.bfloat16)

    xt_cache = {}
    for s, (a, b_) in enumerate(slot_pairs):
        for e in (a, b_):
            xt_cache[e] = xt_for(e, S[s])

    in_maps = []
    for core in range(N_CORES):
        q = core % 4  # this core's F-quarter
        fsl = slice(q * FQ, (q + 1) * FQ)
        im = {}
        b1rows = []
        for s in range(N_SLOTS):
            e = slot_pairs[s][core // 4]  # expert for this core's slot s
            # W1 [2, 128, 8, FQ/2]
            w = w1h[e][:, fsl].reshape(N_CT, 128, 4, FQ // 4).transpose(2, 1, 0, 3)
            im[f"w1{s}"] = np.ascontiguousarray(w)
            # W2 [2, 128, 4, C]
            w = w2h[e][fsl, :].reshape(2, N_FT // 2, 128, C).transpose(0, 2, 1, 3)
            im[f"w2{s}"] = np.ascontiguousarray(w)
            im[f"x{s}t"] = xt_cache[e]
            b1rows.append(
                np.ascontiguousarray(b1[e][fsl].reshape(N_FT, 128).T)
            )
        im["b1t"] = np.stack(b1rows).astype(np.float32)
        im["x0"] = np.ascontiguousarray(
            im["x0t"][:, :, : chunk_lists[0][0]]
        )
        in_maps.append(im)

    nc = build_nc(chunk_lists)
    trace = os.environ.get("KERNEL_TRACE", "") == "1"
    res = run_bass_kernel_spmd(
        nc, in_maps, core_ids=list(range(N_CORES)), trace=trace
    )
    _LAST_RESULTS["bass_results"] = res
    if trace and res.exec_time_ns is not None:
        print(f"[kernel] HW exec time: {res.exec_time_ns} ns")

    out = np.zeros((n_tok_total, C), dtype=np.float32)
    for s in range(N_SLOTS):
        for half, e in enumerate(slot_pairs[s]):
            n_e = counts[e]
            oe = np.zeros((n_e, C), dtype=np.float32)
            for q in range(4):
                core = 4 * half + q
                oe += np.asarray(res.results[core][f"out{s}"])[:n_e].astype(
                    np.float32
                )
            out[idxs[e]] += ws[e][:, None] * (oe + b2[e][None, :])
    return out.reshape(B, T, C)
